# revision 33
# baseline (speedup 1.0000x reference)
"""Distributed Trainium2 kernel: batched multi-head attention.

softmax(Q K^T / sqrt(64)) V for B=2, H=8, S=4096, D=64 (fp32).

Sharding: the 16 (batch, head) slices are split across 8 NeuronCores,
2 heads per core.  Vanilla attention per head needs no cross-core
communication.

Per-core algorithm (per head), per 512-wide q-block:
  S^T[k, q] = sum_d K[k,d] Q[q,d]     (TensorE, bf16 in / fp32 acc,
                                       2-way row-packed pairs on the
                                       d=64 contraction)
  P^T = exp(S^T / 8)                  (split per pair between ScalarE
                                       ACT exact exp and a 1-op VectorE
                                       bit-trick exp2; the two engines
                                       run concurrently, each pair goes
                                       to exactly one of them)
  O[q, 0:65] += P_chunk^T V_aug       (TensorE: P^T q-chunks as the
                                       128x128 stationary operand, V_aug
                                       streams; col 64 of V_aug is ones,
                                       accumulating the softmax
                                       denominator; output lands in
                                       [q, d] layout directly - no
                                       transpose step at all)
  out = O[:, 0:64] * 1/O[:, 64]       (VectorE reciprocal + scalar mul)

The VectorE exp for score s is ONE op: t16 = int16(A*s + B) where
A = 128*log2(e)/8; t16's bits ARE bf16 bits of 2^m*(1+f) (Schraudolph,
used raw).  B is calibrated ~7.4 bits below the exact-bias 16256 so the
multiplicative error (1+f)/2^f is mean-centered (residual ~1.8% RMS,
zero-mean); applied to R_DVE of 16 pairs per block it contributes
~sqrt(R_DVE/16)*1.8% to the output rel-err (softmax numerator and
denominator see the same factors, so the common mode cancels).

Pipeline: a global (block, pair) unit stream through one 3-deep PSUM
score pool.  Units are emitted two at a time — both units' score
matmuls back-to-back (the second pair's two ks LDWEIGHTS hide under
the first pair's 213ns matmul; the PE background weight buffer is one
deep, so an isolated score pair always pays its ~107ns ks load
exposed), then the two pending units' PV matmuls.  DVE pairs sit at
odd positions so each super-step feeds one pair to ScalarE and one to
VectorE concurrently (ACT ~1005ns exact, DVE ~1134ns raw bit-trick;
both inflate ~9% when overlapped — shared PSUM read path).  The
per-block epilogue (reciprocal + 4 normalize muls + group DMA) drains
one op per super-step behind the next block's stream, against a
double-buffered oacc, so VectorE never bursts and the PE never waits
on an epilogue.  A post-build pass prunes semaphore waits that are
transitively implied, which walrus's one-wait-per-instruction limit
would otherwise reject.

Host-side prep (untimed): shard heads, cast to bf16, transpose Q/K to
[d, S] layout (Q duplicated into both partition halves, K packed
even/odd for the 2-way PE row-tiling), append a ones column to V, and
byte-pack everything into ONE fp32 HBM tensor per head.

Walrus in this toolchain allows only ONE semaphore wait per engine
instruction, which shapes several choices:
  - tiny PE "dummy" matmuls (start=False, into spare PSUM columns of
    the live oacc / warmup tile) absorb cross-engine waits before real
    matmuls (NoOps don't credit the engine clock; real Matmults do);
  - input DMAs are split into first-need-ordered pieces (k pairs 0-1,
    q block 0, V tiles 0-3 land first ~3.5us in) on parallel HWDGE
    lanes, each completion absorbed by a dummy right before its first
    consumer; 10 in + 6 out = 16 DMAs on 16 lanes;
  - the Tile kernel-tail gather drain is split into one single-wait
    drain per proc (_SplitDrainTileContext).
"""

import os
import sys
from collections import deque

for _p in ("/opt/trn_rl_repo",):
    if _p not in sys.path:
        sys.path.insert(0, _p)

import numpy as np
import ml_dtypes

import concourse.bass as bass
import concourse.mybir as mybir
from concourse.bass_utils import run_bass_kernel_spmd
from concourse.tile import TileContext
from concourse.tile_sem_assignment import N_PROCS
from concourse.vector_clock import ScopedClock, VectorClock


class _SplitDrainTileContext(TileContext):
    """Emit the kernel-tail gather as one single-wait drain per proc —
    walrus in this toolchain allows only one sync wait per instruction,
    and the stock tail drain carries one wait per active proc."""

    def _drain_and_barrier(self, tick_clock, wait_clock):
        gc = tick_clock.global_clock
        for p in range(N_PROCS):
            if gc[p] == 0:
                continue
            v = [0] * N_PROCS
            v[p] = gc[p]
            d = self.nc.sync.drain()
            wait_clock.add_sem_waits(d.ins, ScopedClock({None: VectorClock(v)}))
        # rest of the stock tail, minus its single multi-wait gather drain
        self.nc.all_engine_barrier()
        assert self.sems is not None
        popped = self.nc._tile_sem_poison_stack.pop()
        assert popped is self._sem_poison
        self.nc.clear_and_free_semaphores(list(self.sems.allocated().values()))
        self.nc.all_engine_barrier()


B, H, S, D = 2, 8, 4096, 64
N_CORES = 8
HPC = (B * H) // N_CORES          # heads per core = 2
NKT = S // 128                    # 32 k-tiles
NPAIR = NKT // 2                  # 16 row-packed pairs
QB = 512                          # q columns per block
NQB = S // QB                     # 8 q blocks
NT = QB // 128                    # 128-wide q chunks per q block = 4
NBLK = HPC * NQB                  # 16 (head, q-block) blocks per core
SCALE = 1.0 / np.sqrt(D)          # folded into the ACT exp / DVE A const

# DVE offload: pairs at these positions of each block's 16-pair loop are
# exp'd on VectorE with the raw 1-op bit-trick.  Tunable via env.
R_DVE = int(os.environ.get("ATTN_R_DVE", "8"))


def d_pos(b: int) -> tuple:
    """DVE pair positions for block b: the ODD positions (from 15 down),
    so each 2-unit emission super-step carries one ACT pair and one DVE
    pair — the two exp engines stay concurrently fed — and pairs 0/1
    stay on ACT for the block start."""
    if not R_DVE:
        return ()
    pos = list(range(15, 0, -2))[:R_DVE]
    if R_DVE > 8:
        pos += list(range(14, 1, -2))[: R_DVE - 8]
    return tuple(sorted(pos))

# DVE bit-trick exp constants: t = A*s + B -> int16; bits are bf16
# (Schraudolph).  B is calibrated below the exact bias 127*128 = 16256
# so the (1+f)/2^f interpolation error (including i16 rounding noise)
# is mean-one: the remaining ~1.8% RMS is zero-mean and uncorrelated
# with V, so it enters the output norm at sqrt(R_DVE/16) weight.
A_DVE = float(SCALE * np.log2(np.e) * 128.0)
_t = np.random.default_rng(0).uniform(-20.0, 20.0, 500_000)
_tau = np.round(128.0 * _t + 16256.0)
_E = np.floor(_tau / 128.0) - 127.0
_r = 2.0 ** _E * (1.0 + (_tau - (_E + 127.0) * 128.0) / 128.0) / 2.0 ** _t
B_DVE = float(16256.0 + 128.0 * np.log2(np.mean(_r) / np.mean(_r * _r)))

# qkv byte-pack layout (fp32 columns per head); Q/K/V all bf16 payload
Q_COLS = S // 2                   # 2048: Q^T (bf16) duplicated in both halves
K_COLS = S // 4                   # 1024: K^T (bf16) even tiles rows 0:64, odd 64:128
VA_F16 = NKT * 66                 # V_aug per k-tile padded to 66 bf16
VA_COLS = VA_F16 // 2             # 1056 fp32
TOT_COLS = Q_COLS + K_COLS + VA_COLS  # 4128

# output DMA groups (q_block ranges) — 3 per head; the tail group is a
# single q-block so the final output DMA is small
OUT_GROUPS = [(0, 4), (4, 7), (7, 8)]

F32 = mybir.dt.float32
BF16 = mybir.dt.bfloat16
I16 = mybir.dt.int16
EXP = mybir.ActivationFunctionType.Exp
ALU = mybir.AluOpType

_built = None
_last_result = None

_ENGINE_PREFIX = {
    mybir.EngineType.PE: "PE",
    mybir.EngineType.Activation: "Activation",
    mybir.EngineType.DVE: "DVE",
}


def _prune_implied_waits(nc: bass.Bass) -> int:
    """Vector-clock replay over the scheduled instruction stream: drop any
    semaphore wait that is transitively implied by (a) the issuing engine's
    in-order history or (b) another wait kept on the same instruction.

    Soundness: PE/ACT/DVE/SP/Pool complete their queues in order (PE matmuls
    are pc-monotone in start and end).  Ldweights is exempt as a waiter (the
    PE pulls it ahead of in-flight matmuls, so queue history is not a valid
    guarantee at its actual execution time).  A DMA transfer's completion
    increment inherits the issuing DMACopy's observed clock (the transfer
    starts no earlier than issue).  Semaphores that ever see a non-increment
    update stop participating.  Walrus allows only ONE sync wait per
    instruction, so Tile's conservative extra waits must go."""
    import bass_rust

    _PRUNE_ALL = bool(int(os.environ.get("ATTN_PRUNE_ALL", "0")))
    _PRUNE_PE = bool(int(os.environ.get("ATTN_PRUNE_PE", "1")))
    pruned = 0
    for fn in nc.m.functions:
        count: dict = {}           # sem id -> total incs so far
        closure: dict = {}         # (sem id, count) -> clock dict at that inc
        eng_clock: dict = {}       # engine -> clock dict (sem id -> count seen)
        dirty: set = set()

        def implied(clk, s, v):
            return clk.get(s, 0) >= v

        for blk in fn.blocks:
            for inst in blk.instructions:
                si = inst.sync_info
                if si is None:
                    continue
                eng = inst.engine
                C = dict(eng_clock.get(eng, {}))
                waits = list(si.on_wait)
                kept = []
                if waits:
                    closures = []
                    for w in waits:
                        cl = dict(closure.get((w.id, w.wait_value), {}))
                        cl[w.id] = max(cl.get(w.id, 0), w.wait_value)
                        closures.append(cl)
                    can_prune = inst.opcode != "Ldweights" and (
                        _PRUNE_ALL
                        or len(waits) >= 2
                        or (_PRUNE_PE and eng == mybir.EngineType.PE)
                    )
                    for i, w in enumerate(waits):
                        if (
                            can_prune
                            and w.sync_type == "semaphore"
                            and w.wait_mode == "sem-ge-imm"
                            and w.id not in dirty
                        ):
                            base = dict(C)
                            for jj, w2 in enumerate(waits):
                                if jj != i and w2 not in kept[:0]:
                                    pass
                            # implied by engine history or any OTHER wait
                            othr = dict(C)
                            for jj in range(len(waits)):
                                if jj == i:
                                    continue
                                for s2, v2 in closures[jj].items():
                                    if othr.get(s2, 0) < v2:
                                        othr[s2] = v2
                            if implied(othr, w.id, w.wait_value):
                                pruned += 1
                                continue
                        kept.append(w)
                    # merge kept waits' closures into the engine clock
                    for w in kept:
                        cl = closure.get((w.id, w.wait_value), {})
                        for s2, v2 in cl.items():
                            if C.get(s2, 0) < v2:
                                C[s2] = v2
                        if C.get(w.id, 0) < w.wait_value:
                            C[w.id] = w.wait_value
                    if len(kept) != len(waits):
                        inst.sync_info = bass_rust.SyncInfo(
                            on_wait=kept, on_update=list(si.on_update)
                        )
                        si = inst.sync_info
                for u in si.on_update:
                    if u.sync_type != "semaphore":
                        continue
                    if u.update_mode in ("sem-inc", "sem-add-imm"):
                        k = count.get(u.id, 0) + u.update_value
                        count[u.id] = k
                        cc = dict(C)
                        cc[u.id] = max(cc.get(u.id, 0), k)
                        closure[(u.id, k)] = cc
                        if C.get(u.id, 0) < k:
                            C[u.id] = k
                    else:
                        dirty.add(u.id)
                eng_clock[eng] = C
    return pruned


def _build_nc() -> bass.Bass:
    nc = bass.Bass()
    qkv_ext = nc.declare_dram_parameter("qkv", [HPC, 128, TOT_COLS], F32, isOutput=False)
    out_ext = nc.declare_dram_parameter("out", [HPC, S, D], F32, isOutput=True)

    _dummy = [None, 0, 0]

    def pe_touch(ap):
        """Tiny PE matmul reading one column of `ap`: absorbs the
        producer's cross-engine wait so later (real) matmuls need at
        most one wait (walrus: 1 sync wait max per Matmult).  Dummies
        write start=False into spare PSUM columns of whatever live tile
        _dummy points at (warm tile pre-loop, the current oacc's 66th
        column in-loop) — no dedicated PSUM bank, so oacc can be
        double-buffered."""
        slot_fn, idx, lim = _dummy
        _dummy[1] += 1
        assert idx < lim, "dummy slots exhausted"
        nc.tensor.matmul(
            slot_fn(idx), lhsT=ap, rhs=ap, start=False, stop=False,
            skip_group_check=True,
        )

    def set_touch_target(slot_fn, nslots):
        _dummy[0] = slot_fn
        _dummy[1] = 0
        _dummy[2] = nslots

    with _SplitDrainTileContext(nc) as tc:
        with (
            tc.tile_pool(name="const", bufs=1) as cpool,
            tc.tile_pool(name="inp", bufs=1) as ipool,
            tc.tile_pool(name="ptp", bufs=8) as ptpool,
            tc.tile_pool(name="ptd", bufs=4 if R_DVE else 1) as ptdpool,
            tc.tile_pool(name="ep", bufs=2) as eppool,
            tc.tile_pool(name="outp", bufs=1) as outpool,
            tc.tile_pool(name="ps_s", bufs=3, space="PSUM") as spool,
            tc.tile_pool(name="ps_o", bufs=2, space="PSUM") as opool,
        ):
            # warm dummy target shares the oacc tag (pools allocate
            # bufs buffers PER TAG) — it cycles out before the 2nd real
            # oacc allocation, costing no extra PSUM bank.
            warm_dmy = opool.tile([128, NT * 66], F32, tag="oacc", name="warm_dmy")
            set_touch_target(lambda k: warm_dmy[0:1, k : k + 1], NT * 66)
            warm = cpool.tile([1, 16], F32, tag="warm", name="warm")
            act_warm = cpool.tile([1, 16], BF16, tag="actw", name="act_warm")
            # Pull the ACT exp table load (~2.7us) to the very front: the
            # input DMA transfers only start once the table-load DMA
            # completes, so every ns it starts earlier is startup saved.
            # memset on VectorE, NOT GpSimd — the GpSimd sequencer inits
            # ~4us later than the others and would hold the table load
            # (and with it all input DMA traffic) hostage.
            nc.vector.memset(warm, 1.0)
            nc.scalar.activation(act_warm, warm, EXP, scale=1.0)
            for _ in range(50):
                pe_touch(warm[0:1, 0:1])

            # Input DMAs: head 0 in 5 pieces on parallel HWDGE lanes so the
            # first score matmuls start ~2.5us in instead of ~10us; head 1
            # in 2 pieces (it has ~100us of slack).  Touches that absorb
            # each piece's completion are emitted where first needed.
            H0_PIECES = [
                ("kt01", Q_COLS, Q_COLS + 128),
                ("qt0", 0, QB // 2),
                ("va03", Q_COLS + K_COLS, Q_COLS + K_COLS + 132),
                ("kt27", Q_COLS + 128, Q_COLS + 448),
                ("varest", Q_COLS + K_COLS + 132, TOT_COLS),
                ("kt8f", Q_COLS + 448, Q_COLS + K_COLS),
                ("qt13", QB // 2, 4 * QB // 2),
                ("qt47", 4 * QB // 2, Q_COLS),
            ]
            qt_sb, kt_sb, va_sb = [], [], []
            dma_regions = {}
            for j in range(HPC):
                qkv = ipool.tile([128, TOT_COLS], F32, tag=f"qkv{j}", name=f"qkv_sb{j}")
                if j == 0:
                    for nm, a, bb in H0_PIECES:
                        nc.sync.dma_start(out=qkv[:, a:bb], in_=qkv_ext[j][:, a:bb])
                        dma_regions[nm] = qkv[0:1, a : a + 1]
                else:
                    nc.sync.dma_start(
                        out=qkv[:, Q_COLS:TOT_COLS], in_=qkv_ext[j][:, Q_COLS:TOT_COLS]
                    )
                    dma_regions[f"h{j}kv"] = qkv[0:1, Q_COLS : Q_COLS + 1]
                    nc.sync.dma_start(out=qkv[:, 0:Q_COLS], in_=qkv_ext[j][:, 0:Q_COLS])
                    dma_regions[f"h{j}q"] = qkv[0:1, 0:1]
                qt_sb.append(qkv[:, 0:Q_COLS].bitcast(BF16))            # [128, S]
                kt_sb.append(qkv[:, Q_COLS : Q_COLS + K_COLS].bitcast(BF16))
                va_sb.append(
                    qkv[:, Q_COLS + K_COLS : TOT_COLS].bitcast(BF16)  # [128, 2112]
                )
            pe_touch(dma_regions["kt01"])
            pe_touch(dma_regions["qt0"])

            # ---- PV: pt q-chunks as stationary operand, V_aug streams;
            #      output accumulates directly in [q, d+1] layout.  One
            #      call emits ONE k-tile (half a pair): the other half
            #      of the pair is emitted after the next unit's score
            #      matmuls so the expensive ks LDWEIGHTS gets a preload
            #      runway under these 4 chunk matmuls.
            def emit_pv_half(j2, p, hh, pt_tile, oacc):
                kt_i = 2 * p + hh
                va_slice = va_sb[j2][:, kt_i * 66 : kt_i * 66 + 65]
                for c in range(NT):
                    # start=True clears has_written for the WHOLE bank,
                    # so only the very first chunk-MM may carry it; the
                    # other chunks' first writes then overwrite (their
                    # has_written was cleared by that same bank clear)
                    # and later writes accumulate.
                    nc.tensor.matmul(
                        oacc[:, c, 0:65],
                        lhsT=pt_tile[:, hh * QB + c * 128 : hh * QB + (c + 1) * 128],
                        rhs=va_slice,
                        start=(p == 0 and hh == 0 and c == 0),
                        stop=(p == NPAIR - 1 and hh == 1),
                        skip_group_check=True,
                    )

            # ---- global (block, pair) stream, software-pipelined:
            # scores+exp for pair i are emitted LOOK pairs ahead of that
            # pair's PV matmuls, so ScalarE and VectorE always hold >=2
            # score tiles to exp CONCURRENTLY while the PE works through
            # PV.  LOOK = spool bufs - 1 (the PSUM slot for pair i frees
            # when exp(i) completes, which PV(i) already waited out).
            # The per-block epilogue (4 normalize muls + out-group DMA)
            # is DEFERRED one step at a time into the following block's
            # stream so it never bursts the VectorE queue (oacc is
            # double-buffered, so block b's oacc stays readable while
            # block b+1 accumulates).
            LOOK = 2
            ot_g = [None]
            oacc_ref = [None]
            deferred = deque()

            def emit_S(b, p):
                j, qb = divmod(b, NQB)
                qs = qt_sb[j][:, qb * QB : (qb + 1) * QB]
                ks = kt_sb[j][:, p * 128 : (p + 1) * 128]
                s_pair = spool.tile([128, 2 * QB], F32, tag="s", name="s_pair")
                nc.tensor.matmul(
                    s_pair[:, 0:QB], lhsT=ks[0:64, :], rhs=qs[0:64, :],
                    start=True, stop=True,
                )
                nc.tensor.matmul(
                    s_pair[:, QB : 2 * QB], lhsT=ks[64:128, :], rhs=qs[64:128, :],
                    start=True, stop=True,
                )
                if p in d_pos(b):
                    t16 = ptdpool.tile([128, 2 * QB], I16, tag="t16", name="t16")
                    nc.vector.tensor_scalar(
                        out=t16, in0=s_pair, scalar1=A_DVE, scalar2=B_DVE,
                        op0=ALU.mult, op1=ALU.add,
                    )
                    return t16.bitcast(BF16)
                pt = ptpool.tile([128, 2 * QB], BF16, tag="pt", name="pt")
                nc.scalar.activation(pt, s_pair, EXP, scale=float(SCALE))
                return pt

            def pv_prologue(b, p):
                j, qb = divmod(b, NQB)
                gi = next(i for i, (a, e) in enumerate(OUT_GROUPS) if a <= qb < e)
                g0, g1_ = OUT_GROUPS[gi]
                if p == 0:
                    if qb == g0:
                        ot_g[0] = outpool.tile(
                            [128, (g1_ - g0) * NT, 64], F32,
                            tag=f"ot{j}_{gi}", name=f"ot{j}_{gi}",
                        )
                    oacc_ref[0] = opool.tile(
                        [128, NT, 66], F32, tag="oacc", name="oacc"
                    )
                    set_touch_target(
                        lambda k, o=oacc_ref[0]: o[
                            32 * (k // NT) : 32 * (k // NT) + 1, k % NT, 65:66
                        ],
                        4 * NT,
                    )
                    if b == 0:
                        pe_touch(dma_regions["va03"])

            def pv_epilogue(b):
                # normalize [q, d] chunks straight out of PSUM; the recip
                # is emitted now (140ns on DVE), the 4 muls + group DMA
                # drain one per stream step behind the next block's work.
                j, qb = divmod(b, NQB)
                gi = next(i for i, (a, e) in enumerate(OUT_GROUPS) if a <= qb < e)
                g0, g1_ = OUT_GROUPS[gi]
                oacc = oacc_ref[0]
                otg = ot_g[0]
                recip = eppool.tile([128, NT], F32, tag="recip", name="recip")

                def run_recip():
                    nc.vector.reciprocal(recip, oacc[:, :, 64])
                deferred.append(run_recip)

                def mk_mul(t):
                    # alternate normalize muls between VectorE and the
                    # ScalarE Copy-with-scale path so neither exp lane
                    # takes the whole epilogue on top of its exp stream
                    def run_dve():
                        nc.vector.tensor_scalar_mul(
                            otg[:, (qb - g0) * NT + t, :],
                            oacc[:, t, 0:64],
                            recip[:, t : t + 1],
                        )

                    def run_act():
                        nc.scalar.activation(
                            otg[:, (qb - g0) * NT + t, :],
                            oacc[:, t, 0:64],
                            mybir.ActivationFunctionType.Copy,
                            scale=recip[:, t : t + 1],
                        )

                    return run_act if True else run_dve

                for t in range(NT):
                    deferred.append(mk_mul(t))
                if qb == g1_ - 1:
                    def run_dma():
                        nc.sync.dma_start(
                            out=out_ext[j, g0 * QB : g1_ * QB, :].rearrange(
                                "(t p) d -> p t d", p=128
                            ),
                            in_=otg,
                        )
                    deferred.append(run_dma)

            # Units are emitted two at a time: both units' score matmuls
            # go back-to-back (the second pair's two ks LDWEIGHTS hide
            # entirely under the first pair's 213ns matmul — the PE's
            # single background weight buffer can only run ONE load
            # ahead, so an isolated score pair always pays its ks load
            # exposed), then both pending units' PV matmuls.
            NUNITS = NBLK * NPAIR
            pend = deque()
            ui = 0
            fresh_epi = False
            while ui < NUNITS or pend:
                for _ in range(2):
                    if ui < NUNITS:
                        b, p = divmod(ui, NPAIR)
                        if b == 0 and p == 2:
                            pe_touch(dma_regions["kt27"])
                        if b == 0 and p == 7:
                            pe_touch(dma_regions["kt8f"])
                        if b == 1 and p == 0:
                            pe_touch(dma_regions["qt13"])
                        if b == 4 and p == 0:
                            pe_touch(dma_regions["qt47"])
                        pend.append((b, p, emit_S(b, p)))
                        ui += 1
                first_pop = True
                while len(pend) > (LOOK if ui < NUNITS else 0):
                    bb_, pp_, pt_ = pend.popleft()
                    jj_ = bb_ // NQB
                    pv_prologue(bb_, pp_)
                    if first_pop and ui > 4:
                        # 1x1 filler matmul at the score->PV boundary:
                        # it soaks the post-score array-drain window so
                        # the first real PV chunk matmul streams clean
                        pe_touch(warm[0:1, 0:1])
                        first_pop = False
                    if bb_ == 0 and pp_ == 2:
                        pe_touch(dma_regions["varest"])
                    if bb_ == 3 and pp_ == 2:
                        for nm in dma_regions:
                            if nm.startswith("h"):
                                pe_touch(dma_regions[nm])
                    emit_pv_half(jj_, pp_, 0, pt_, oacc_ref[0])
                    emit_pv_half(jj_, pp_, 1, pt_, oacc_ref[0])
                    if pp_ == NPAIR - 1:
                        pv_epilogue(bb_)
                        fresh_epi = True
                if deferred and not fresh_epi:
                    deferred.popleft()()
                fresh_epi = False
            while deferred:
                deferred.popleft()()
    n = _prune_implied_waits(nc)
    print(f'[kernel] pruned {n} implied waits', file=sys.stderr)
    return nc


def _get_nc():
    global _built
    if _built is None:
        _built = _build_nc()
    return _built


def _pack_head(q_head: np.ndarray, k_head: np.ndarray, v_head: np.ndarray) -> np.ndarray:
    """Build the per-head [128, TOT_COLS] fp32 input block (bf16 payload)."""
    qt = np.ascontiguousarray(q_head.T).astype(ml_dtypes.bfloat16)  # [64, S]
    qt2 = np.concatenate([qt, qt], axis=0)                  # [128, S]

    kt = np.ascontiguousarray(k_head.T).astype(ml_dtypes.bfloat16).reshape(64, NKT, 128)
    ktp = np.concatenate(
        [kt[:, 0::2].reshape(64, -1), kt[:, 1::2].reshape(64, -1)], axis=0
    )                                                       # [128, S/2]

    va = np.zeros((128, NKT, 66), dtype=ml_dtypes.bfloat16)
    va[:, :, :64] = v_head.reshape(NKT, 128, 64).transpose(1, 0, 2)
    va[:, :, 64] = 1.0

    return np.concatenate(
        [
            qt2.view(np.float32),
            ktp.view(np.float32),
            va.reshape(128, -1).view(np.float32),
        ],
        axis=1,
    )                                                       # [128, TOT_COLS]


def kernel(Q: np.ndarray, K: np.ndarray, V: np.ndarray) -> np.ndarray:
    global _last_result
    Q = np.asarray(Q, dtype=np.float32).reshape(B * H, S, D)
    K = np.asarray(K, dtype=np.float32).reshape(B * H, S, D)
    V = np.asarray(V, dtype=np.float32).reshape(B * H, S, D)

    in_maps = []
    for c in range(N_CORES):
        heads = range(c * HPC, (c + 1) * HPC)
        in_maps.append(
            {"qkv": np.stack([_pack_head(Q[h], K[h], V[h]) for h in heads])}
        )

    nc = _get_nc()
    trace = bool(int(os.environ.get("ATTN_TRACE", "0")))
    res = run_bass_kernel_spmd(
        nc, in_maps, core_ids=list(range(N_CORES)), trace=trace
    )
    _last_result = res

    out = np.empty((B * H, S, D), dtype=np.float32)
    for c in range(N_CORES):
        out[c * HPC : (c + 1) * HPC] = res.results[c]["out"]
    return out.reshape(B, H, S, D)



# revision 34
# speedup vs baseline: 1.0310x; 1.0310x over previous
"""Distributed Trainium2 kernel: batched multi-head attention.

softmax(Q K^T / sqrt(64)) V for B=2, H=8, S=4096, D=64 (fp32).

Sharding: the 16 (batch, head) slices are split across 8 NeuronCores,
2 heads per core.  Vanilla attention per head needs no cross-core
communication.

Per-core algorithm (per head), per 512-wide q-block:
  S^T[k, q] = sum_d K[k,d] Q[q,d]     (TensorE, bf16 in / fp32 acc,
                                       2-way row-packed pairs on the
                                       d=64 contraction)
  P^T = exp(S^T / 8)                  (split per pair between ScalarE
                                       ACT exact exp and a 1-op VectorE
                                       bit-trick exp2; the two engines
                                       run concurrently, each pair goes
                                       to exactly one of them)
  O[q, 0:65] += P_chunk^T V_aug       (TensorE: P^T q-chunks as the
                                       128x128 stationary operand, V_aug
                                       streams; col 64 of V_aug is ones,
                                       accumulating the softmax
                                       denominator; output lands in
                                       [q, d] layout directly - no
                                       transpose step at all)
  out = O[:, 0:64] * 1/O[:, 64]       (VectorE reciprocal + scalar mul)

The VectorE exp for score s is ONE op: t16 = int16(A*s + B) where
A = 128*log2(e)/8; t16's bits ARE bf16 bits of 2^m*(1+f) (Schraudolph,
used raw).  B is calibrated ~7.4 bits below the exact-bias 16256 so the
multiplicative error (1+f)/2^f is mean-centered (residual ~1.8% RMS,
zero-mean); applied to R_DVE of 16 pairs per block it contributes
~sqrt(R_DVE/16)*1.8% to the output rel-err (softmax numerator and
denominator see the same factors, so the common mode cancels).

Pipeline: a global (block, pair) unit stream through one 3-deep PSUM
score pool.  Units are emitted two at a time — both units' score
matmuls back-to-back (the second pair's two ks LDWEIGHTS hide under
the first pair's 213ns matmul; the PE background weight buffer is one
deep, so an isolated score pair always pays its ~107ns ks load
exposed), then the two pending units' PV matmuls.  DVE pairs sit at
odd positions so each super-step feeds one pair to ScalarE and one to
VectorE concurrently (ACT ~1005ns exact, DVE ~1134ns raw bit-trick;
both inflate ~9% when overlapped — shared PSUM read path).  The
per-block epilogue (reciprocal + 4 normalize muls + group DMA) drains
one op per super-step behind the next block's stream, against a
double-buffered oacc, so VectorE never bursts and the PE never waits
on an epilogue.  A post-build pass prunes semaphore waits that are
transitively implied, which walrus's one-wait-per-instruction limit
would otherwise reject.

Host-side prep (untimed): shard heads, cast to bf16, transpose Q/K to
[d, S] layout (Q duplicated into both partition halves, K packed
even/odd for the 2-way PE row-tiling), append a ones column to V, and
byte-pack everything into ONE fp32 HBM tensor per head.

Walrus in this toolchain allows only ONE semaphore wait per engine
instruction, which shapes several choices:
  - tiny PE "dummy" matmuls (start=False, into spare PSUM columns of
    the live oacc / warmup tile) absorb cross-engine waits before real
    matmuls (NoOps don't credit the engine clock; real Matmults do);
  - input DMAs are split into first-need-ordered pieces (k pairs 0-1,
    q block 0, V tiles 0-3 land first ~3.5us in) on parallel HWDGE
    lanes, each completion absorbed by a dummy right before its first
    consumer; 10 in + 6 out = 16 DMAs on 16 lanes;
  - the Tile kernel-tail gather drain is split into one single-wait
    drain per proc (_SplitDrainTileContext).
"""

import os
import sys
from collections import deque

for _p in ("/opt/trn_rl_repo",):
    if _p not in sys.path:
        sys.path.insert(0, _p)

import numpy as np
import ml_dtypes

import concourse.bass as bass
import concourse.mybir as mybir
from concourse.bass_utils import run_bass_kernel_spmd
from concourse.tile import TileContext
from concourse.tile_sem_assignment import N_PROCS
from concourse.vector_clock import ScopedClock, VectorClock


class _SplitDrainTileContext(TileContext):
    """Emit the kernel-tail gather as one single-wait drain per proc —
    walrus in this toolchain allows only one sync wait per instruction,
    and the stock tail drain carries one wait per active proc."""

    def _drain_and_barrier(self, tick_clock, wait_clock):
        gc = tick_clock.global_clock
        for p in range(N_PROCS):
            if gc[p] == 0:
                continue
            v = [0] * N_PROCS
            v[p] = gc[p]
            d = self.nc.sync.drain()
            wait_clock.add_sem_waits(d.ins, ScopedClock({None: VectorClock(v)}))
        # rest of the stock tail, minus its single multi-wait gather drain
        self.nc.all_engine_barrier()
        assert self.sems is not None
        popped = self.nc._tile_sem_poison_stack.pop()
        assert popped is self._sem_poison
        self.nc.clear_and_free_semaphores(list(self.sems.allocated().values()))
        self.nc.all_engine_barrier()


B, H, S, D = 2, 8, 4096, 64
N_CORES = 8
HPC = (B * H) // N_CORES          # heads per core = 2
NKT = S // 128                    # 32 k-tiles
NPAIR = NKT // 2                  # 16 row-packed pairs
QB = 512                          # q columns per block
NQB = S // QB                     # 8 q blocks
NT = QB // 128                    # 128-wide q chunks per q block = 4
NBLK = HPC * NQB                  # 16 (head, q-block) blocks per core
SCALE = 1.0 / np.sqrt(D)          # folded into the ACT exp / DVE A const

# DVE offload: pairs at these positions of each block's 16-pair loop are
# exp'd on VectorE with the raw 1-op bit-trick.  Tunable via env.
R_DVE = int(os.environ.get("ATTN_R_DVE", "8"))


def d_pos(b: int) -> tuple:
    """DVE pair positions for block b: the ODD positions (from 15 down),
    so each 2-unit emission super-step carries one ACT pair and one DVE
    pair — the two exp engines stay concurrently fed — and pairs 0/1
    stay on ACT for the block start."""
    if not R_DVE:
        return ()
    pos = list(range(15, 0, -2))[:R_DVE]
    if R_DVE > 8:
        pos += list(range(14, 1, -2))[: R_DVE - 8]
    return tuple(sorted(pos))

# DVE bit-trick exp constants: t = A*s + B -> int16; bits are bf16
# (Schraudolph).  B is calibrated below the exact bias 127*128 = 16256
# so the (1+f)/2^f interpolation error (including i16 rounding noise)
# is mean-one: the remaining ~1.8% RMS is zero-mean and uncorrelated
# with V, so it enters the output norm at sqrt(R_DVE/16) weight.
A_DVE = float(SCALE * np.log2(np.e) * 128.0)
_t = np.random.default_rng(0).uniform(-20.0, 20.0, 500_000)
_tau = np.round(128.0 * _t + 16256.0)
_E = np.floor(_tau / 128.0) - 127.0
_r = 2.0 ** _E * (1.0 + (_tau - (_E + 127.0) * 128.0) / 128.0) / 2.0 ** _t
B_DVE = float(16256.0 + 128.0 * np.log2(np.mean(_r) / np.mean(_r * _r)))

# qkv byte-pack layout (fp32 columns per head); Q/K/V all bf16 payload
Q_COLS = S // 2                   # 2048: Q^T (bf16) duplicated in both halves
K_COLS = S // 4                   # 1024: K^T (bf16) even tiles rows 0:64, odd 64:128
VA_F16 = NKT * 66                 # V_aug per k-tile padded to 66 bf16
VA_COLS = VA_F16 // 2             # 1056 fp32
TOT_COLS = Q_COLS + K_COLS + VA_COLS  # 4128

# output DMA groups (q_block ranges) — 3 per head; the tail group is a
# single q-block so the final output DMA is small
OUT_GROUPS = [(0, 4), (4, 7), (7, 8)]

F32 = mybir.dt.float32
BF16 = mybir.dt.bfloat16
I16 = mybir.dt.int16
EXP = mybir.ActivationFunctionType.Exp
ALU = mybir.AluOpType

_built = None
_last_result = None

_ENGINE_PREFIX = {
    mybir.EngineType.PE: "PE",
    mybir.EngineType.Activation: "Activation",
    mybir.EngineType.DVE: "DVE",
}


def _prune_implied_waits(nc: bass.Bass) -> int:
    """Vector-clock replay over the scheduled instruction stream: drop any
    semaphore wait that is transitively implied by (a) the issuing engine's
    in-order history or (b) another wait kept on the same instruction.

    Soundness: PE/ACT/DVE/SP/Pool complete their queues in order (PE matmuls
    are pc-monotone in start and end).  Ldweights is exempt as a waiter (the
    PE pulls it ahead of in-flight matmuls, so queue history is not a valid
    guarantee at its actual execution time).  A DMA transfer's completion
    increment inherits the issuing DMACopy's observed clock (the transfer
    starts no earlier than issue).  Semaphores that ever see a non-increment
    update stop participating.  Walrus allows only ONE sync wait per
    instruction, so Tile's conservative extra waits must go."""
    import bass_rust

    _PRUNE_ALL = bool(int(os.environ.get("ATTN_PRUNE_ALL", "0")))
    _PRUNE_PE = bool(int(os.environ.get("ATTN_PRUNE_PE", "1")))
    pruned = 0
    for fn in nc.m.functions:
        count: dict = {}           # sem id -> total incs so far
        closure: dict = {}         # (sem id, count) -> clock dict at that inc
        eng_clock: dict = {}       # engine -> clock dict (sem id -> count seen)
        dirty: set = set()

        def implied(clk, s, v):
            return clk.get(s, 0) >= v

        for blk in fn.blocks:
            for inst in blk.instructions:
                si = inst.sync_info
                if si is None:
                    continue
                eng = inst.engine
                C = dict(eng_clock.get(eng, {}))
                waits = list(si.on_wait)
                kept = []
                if waits:
                    closures = []
                    for w in waits:
                        cl = dict(closure.get((w.id, w.wait_value), {}))
                        cl[w.id] = max(cl.get(w.id, 0), w.wait_value)
                        closures.append(cl)
                    can_prune = inst.opcode != "Ldweights" and (
                        _PRUNE_ALL
                        or len(waits) >= 2
                        or (_PRUNE_PE and eng == mybir.EngineType.PE)
                    )
                    for i, w in enumerate(waits):
                        if (
                            can_prune
                            and w.sync_type == "semaphore"
                            and w.wait_mode == "sem-ge-imm"
                            and w.id not in dirty
                        ):
                            base = dict(C)
                            for jj, w2 in enumerate(waits):
                                if jj != i and w2 not in kept[:0]:
                                    pass
                            # implied by engine history or any OTHER wait
                            othr = dict(C)
                            for jj in range(len(waits)):
                                if jj == i:
                                    continue
                                for s2, v2 in closures[jj].items():
                                    if othr.get(s2, 0) < v2:
                                        othr[s2] = v2
                            if implied(othr, w.id, w.wait_value):
                                pruned += 1
                                continue
                        kept.append(w)
                    # merge kept waits' closures into the engine clock
                    for w in kept:
                        cl = closure.get((w.id, w.wait_value), {})
                        for s2, v2 in cl.items():
                            if C.get(s2, 0) < v2:
                                C[s2] = v2
                        if C.get(w.id, 0) < w.wait_value:
                            C[w.id] = w.wait_value
                    if len(kept) != len(waits):
                        inst.sync_info = bass_rust.SyncInfo(
                            on_wait=kept, on_update=list(si.on_update)
                        )
                        si = inst.sync_info
                for u in si.on_update:
                    if u.sync_type != "semaphore":
                        continue
                    if u.update_mode in ("sem-inc", "sem-add-imm"):
                        k = count.get(u.id, 0) + u.update_value
                        count[u.id] = k
                        cc = dict(C)
                        cc[u.id] = max(cc.get(u.id, 0), k)
                        closure[(u.id, k)] = cc
                        if C.get(u.id, 0) < k:
                            C[u.id] = k
                    else:
                        dirty.add(u.id)
                eng_clock[eng] = C
    return pruned


def _build_nc() -> bass.Bass:
    nc = bass.Bass()
    qkv_ext = nc.declare_dram_parameter("qkv", [HPC, 128, TOT_COLS], F32, isOutput=False)
    out_ext = nc.declare_dram_parameter("out", [HPC, S, D], F32, isOutput=True)

    _dummy = [None, 0, 0]

    def pe_touch(ap):
        """Tiny PE matmul reading one column of `ap`: absorbs the
        producer's cross-engine wait so later (real) matmuls need at
        most one wait (walrus: 1 sync wait max per Matmult).  Dummies
        write start=False into spare PSUM columns of whatever live tile
        _dummy points at (warm tile pre-loop, the current oacc's 66th
        column in-loop) — no dedicated PSUM bank, so oacc can be
        double-buffered."""
        slot_fn, idx, lim = _dummy
        _dummy[1] += 1
        assert idx < lim, "dummy slots exhausted"
        nc.tensor.matmul(
            slot_fn(idx), lhsT=ap, rhs=ap, start=False, stop=False,
            skip_group_check=True,
        )

    def set_touch_target(slot_fn, nslots):
        _dummy[0] = slot_fn
        _dummy[1] = 0
        _dummy[2] = nslots

    with _SplitDrainTileContext(nc) as tc:
        with (
            tc.tile_pool(name="const", bufs=1) as cpool,
            tc.tile_pool(name="inp", bufs=1) as ipool,
            tc.tile_pool(name="ptp", bufs=8) as ptpool,
            tc.tile_pool(name="ptd", bufs=4 if R_DVE else 1) as ptdpool,
            tc.tile_pool(name="ep", bufs=2) as eppool,
            tc.tile_pool(name="outp", bufs=1) as outpool,
            tc.tile_pool(name="ps_s", bufs=3, space="PSUM") as spool,
            tc.tile_pool(name="ps_o", bufs=2, space="PSUM") as opool,
        ):
            # warm dummy target shares the oacc tag (pools allocate
            # bufs buffers PER TAG) — it cycles out before the 2nd real
            # oacc allocation, costing no extra PSUM bank.
            warm_dmy = opool.tile([128, NT * 66], F32, tag="oacc", name="warm_dmy")
            set_touch_target(lambda k: warm_dmy[0:1, k : k + 1], NT * 66)
            warm = cpool.tile([1, 16], F32, tag="warm", name="warm")
            act_warm = cpool.tile([1, 16], BF16, tag="actw", name="act_warm")
            # Pull the ACT exp table load (~2.7us) to the very front: the
            # input DMA transfers only start once the table-load DMA
            # completes, so every ns it starts earlier is startup saved.
            # memset on VectorE, NOT GpSimd — the GpSimd sequencer inits
            # ~4us later than the others and would hold the table load
            # (and with it all input DMA traffic) hostage.
            nc.vector.memset(warm, 1.0)
            nc.scalar.activation(act_warm, warm, EXP, scale=1.0)
            for _ in range(50):
                pe_touch(warm[0:1, 0:1])

            # Input DMAs: head 0 in 5 pieces on parallel HWDGE lanes so the
            # first score matmuls start ~2.5us in instead of ~10us; head 1
            # in 2 pieces (it has ~100us of slack).  Touches that absorb
            # each piece's completion are emitted where first needed.
            H0_PIECES = [
                ("kt01", Q_COLS, Q_COLS + 128),
                ("qt0", 0, QB // 2),
                ("va03", Q_COLS + K_COLS, Q_COLS + K_COLS + 132),
                ("kt27", Q_COLS + 128, Q_COLS + 448),
                ("varest", Q_COLS + K_COLS + 132, TOT_COLS),
                ("kt8f", Q_COLS + 448, Q_COLS + K_COLS),
                ("qt13", QB // 2, 4 * QB // 2),
                ("qt47", 4 * QB // 2, Q_COLS),
            ]
            qt_sb, kt_sb, va_sb = [], [], []
            dma_regions = {}
            for j in range(HPC):
                qkv = ipool.tile([128, TOT_COLS], F32, tag=f"qkv{j}", name=f"qkv_sb{j}")
                if j == 0:
                    for nm, a, bb in H0_PIECES:
                        nc.sync.dma_start(out=qkv[:, a:bb], in_=qkv_ext[j][:, a:bb])
                        dma_regions[nm] = qkv[0:1, a : a + 1]
                else:
                    nc.sync.dma_start(
                        out=qkv[:, Q_COLS:TOT_COLS], in_=qkv_ext[j][:, Q_COLS:TOT_COLS]
                    )
                    dma_regions[f"h{j}kv"] = qkv[0:1, Q_COLS : Q_COLS + 1]
                    nc.sync.dma_start(out=qkv[:, 0:Q_COLS], in_=qkv_ext[j][:, 0:Q_COLS])
                    dma_regions[f"h{j}q"] = qkv[0:1, 0:1]
                qt_sb.append(qkv[:, 0:Q_COLS].bitcast(BF16))            # [128, S]
                kt_sb.append(qkv[:, Q_COLS : Q_COLS + K_COLS].bitcast(BF16))
                va_sb.append(
                    qkv[:, Q_COLS + K_COLS : TOT_COLS].bitcast(BF16)  # [128, 2112]
                )
            pe_touch(dma_regions["kt01"])
            pe_touch(dma_regions["qt0"])

            # ---- PV: pt q-chunks as stationary operand, V_aug streams;
            #      output accumulates directly in [q, d+1] layout.  One
            #      call emits ONE k-tile (half a pair): the other half
            #      of the pair is emitted after the next unit's score
            #      matmuls so the expensive ks LDWEIGHTS gets a preload
            #      runway under these 4 chunk matmuls.
            def emit_pv_half(j2, p, hh, pt_tile, oacc):
                kt_i = 2 * p + hh
                va_slice = va_sb[j2][:, kt_i * 66 : kt_i * 66 + 65]
                for c in range(NT):
                    # start=True clears has_written for the WHOLE bank,
                    # so only the very first chunk-MM may carry it; the
                    # other chunks' first writes then overwrite (their
                    # has_written was cleared by that same bank clear)
                    # and later writes accumulate.
                    nc.tensor.matmul(
                        oacc[:, c, 0:65],
                        lhsT=pt_tile[:, hh * QB + c * 128 : hh * QB + (c + 1) * 128],
                        rhs=va_slice,
                        start=(p == 0 and hh == 0 and c == 0),
                        stop=(p == NPAIR - 1 and hh == 1),
                        skip_group_check=True,
                    )

            # ---- global (block, pair) stream, software-pipelined:
            # scores+exp for pair i are emitted LOOK pairs ahead of that
            # pair's PV matmuls, so ScalarE and VectorE always hold >=2
            # score tiles to exp CONCURRENTLY while the PE works through
            # PV.  LOOK = spool bufs - 1 (the PSUM slot for pair i frees
            # when exp(i) completes, which PV(i) already waited out).
            # The per-block epilogue (4 normalize muls + out-group DMA)
            # is DEFERRED one step at a time into the following block's
            # stream so it never bursts the VectorE queue (oacc is
            # double-buffered, so block b's oacc stays readable while
            # block b+1 accumulates).
            LOOK = 2
            ot_g = [None]
            oacc_ref = [None]
            deferred = deque()

            def emit_S(b, p):
                j, qb = divmod(b, NQB)
                qs = qt_sb[j][:, qb * QB : (qb + 1) * QB]
                ks = kt_sb[j][:, p * 128 : (p + 1) * 128]
                s_pair = spool.tile([128, 2 * QB], F32, tag="s", name="s_pair")
                nc.tensor.matmul(
                    s_pair[:, 0:QB], lhsT=ks[0:64, :], rhs=qs[0:64, :],
                    start=True, stop=True,
                )
                nc.tensor.matmul(
                    s_pair[:, QB : 2 * QB], lhsT=ks[64:128, :], rhs=qs[64:128, :],
                    start=True, stop=True,
                )
                if p in d_pos(b):
                    t16 = ptdpool.tile([128, 2 * QB], I16, tag="t16", name="t16")
                    nc.vector.tensor_scalar(
                        out=t16, in0=s_pair, scalar1=A_DVE, scalar2=B_DVE,
                        op0=ALU.mult, op1=ALU.add,
                    )
                    return t16.bitcast(BF16)
                pt = ptpool.tile([128, 2 * QB], BF16, tag="pt", name="pt")
                nc.scalar.activation(pt, s_pair, EXP, scale=float(SCALE))
                return pt

            def pv_prologue(b, p):
                j, qb = divmod(b, NQB)
                gi = next(i for i, (a, e) in enumerate(OUT_GROUPS) if a <= qb < e)
                g0, g1_ = OUT_GROUPS[gi]
                if p == 0:
                    if qb == g0:
                        ot_g[0] = outpool.tile(
                            [128, (g1_ - g0) * NT, 64], F32,
                            tag=f"ot{j}_{gi}", name=f"ot{j}_{gi}",
                        )
                    oacc_ref[0] = opool.tile(
                        [128, NT, 66], F32, tag="oacc", name="oacc"
                    )
                    set_touch_target(
                        lambda k, o=oacc_ref[0]: o[0:1, k, 65:66], NT
                    )
                    if b == 0:
                        pe_touch(dma_regions["va03"])

            def pv_epilogue(b):
                # normalize [q, d] chunks straight out of PSUM; the recip
                # is emitted now (140ns on DVE), the 4 muls + group DMA
                # drain one per stream step behind the next block's work.
                j, qb = divmod(b, NQB)
                gi = next(i for i, (a, e) in enumerate(OUT_GROUPS) if a <= qb < e)
                g0, g1_ = OUT_GROUPS[gi]
                oacc = oacc_ref[0]
                otg = ot_g[0]
                recip = eppool.tile([128, NT], F32, tag="recip", name="recip")

                def run_recip():
                    nc.vector.reciprocal(recip, oacc[:, :, 64])
                deferred.append(run_recip)

                def mk_mul(t):
                    # alternate normalize muls between VectorE and the
                    # ScalarE Copy-with-scale path so neither exp lane
                    # takes the whole epilogue on top of its exp stream
                    def run_dve():
                        nc.vector.tensor_scalar_mul(
                            otg[:, (qb - g0) * NT + t, :],
                            oacc[:, t, 0:64],
                            recip[:, t : t + 1],
                        )

                    def run_act():
                        nc.scalar.activation(
                            otg[:, (qb - g0) * NT + t, :],
                            oacc[:, t, 0:64],
                            mybir.ActivationFunctionType.Copy,
                            scale=recip[:, t : t + 1],
                        )

                    return run_act if True else run_dve

                for t in range(NT):
                    deferred.append(mk_mul(t))
                if qb == g1_ - 1:
                    def run_dma():
                        nc.sync.dma_start(
                            out=out_ext[j, g0 * QB : g1_ * QB, :].rearrange(
                                "(t p) d -> p t d", p=128
                            ),
                            in_=otg,
                        )
                    deferred.append(run_dma)

            # Units are emitted two at a time: both units' score matmuls
            # go back-to-back (the second pair's two ks LDWEIGHTS hide
            # entirely under the first pair's 213ns matmul — the PE's
            # single background weight buffer can only run ONE load
            # ahead, so an isolated score pair always pays its ks load
            # exposed), then both pending units' PV matmuls.
            NUNITS = NBLK * NPAIR
            pend = deque()
            ui = 0
            fresh_epi = False
            while ui < NUNITS or pend:
                for _ in range(2):
                    if ui < NUNITS:
                        b, p = divmod(ui, NPAIR)
                        if b == 0 and p == 2:
                            pe_touch(dma_regions["kt27"])
                        if b == 0 and p == 7:
                            pe_touch(dma_regions["kt8f"])
                        if b == 1 and p == 0:
                            pe_touch(dma_regions["qt13"])
                        if b == 4 and p == 0:
                            pe_touch(dma_regions["qt47"])
                        pend.append((b, p, emit_S(b, p)))
                        ui += 1
                while len(pend) > (LOOK if ui < NUNITS else 0):
                    bb_, pp_, pt_ = pend.popleft()
                    jj_ = bb_ // NQB
                    pv_prologue(bb_, pp_)
                    if bb_ == 0 and pp_ == 2:
                        pe_touch(dma_regions["varest"])
                    if bb_ == 3 and pp_ == 2:
                        for nm in dma_regions:
                            if nm.startswith("h"):
                                pe_touch(dma_regions[nm])
                    emit_pv_half(jj_, pp_, 0, pt_, oacc_ref[0])
                    emit_pv_half(jj_, pp_, 1, pt_, oacc_ref[0])
                    if pp_ == NPAIR - 1:
                        pv_epilogue(bb_)
                        fresh_epi = True
                if deferred and not fresh_epi:
                    deferred.popleft()()
                fresh_epi = False
            while deferred:
                deferred.popleft()()
    n = _prune_implied_waits(nc)
    print(f'[kernel] pruned {n} implied waits', file=sys.stderr)
    return nc


def _get_nc():
    global _built
    if _built is None:
        _built = _build_nc()
    return _built


def _pack_head(q_head: np.ndarray, k_head: np.ndarray, v_head: np.ndarray) -> np.ndarray:
    """Build the per-head [128, TOT_COLS] fp32 input block (bf16 payload)."""
    qt = np.ascontiguousarray(q_head.T).astype(ml_dtypes.bfloat16)  # [64, S]
    qt2 = np.concatenate([qt, qt], axis=0)                  # [128, S]

    kt = np.ascontiguousarray(k_head.T).astype(ml_dtypes.bfloat16).reshape(64, NKT, 128)
    ktp = np.concatenate(
        [kt[:, 0::2].reshape(64, -1), kt[:, 1::2].reshape(64, -1)], axis=0
    )                                                       # [128, S/2]

    va = np.zeros((128, NKT, 66), dtype=ml_dtypes.bfloat16)
    va[:, :, :64] = v_head.reshape(NKT, 128, 64).transpose(1, 0, 2)
    va[:, :, 64] = 1.0

    return np.concatenate(
        [
            qt2.view(np.float32),
            ktp.view(np.float32),
            va.reshape(128, -1).view(np.float32),
        ],
        axis=1,
    )                                                       # [128, TOT_COLS]


def kernel(Q: np.ndarray, K: np.ndarray, V: np.ndarray) -> np.ndarray:
    global _last_result
    Q = np.asarray(Q, dtype=np.float32).reshape(B * H, S, D)
    K = np.asarray(K, dtype=np.float32).reshape(B * H, S, D)
    V = np.asarray(V, dtype=np.float32).reshape(B * H, S, D)

    in_maps = []
    for c in range(N_CORES):
        heads = range(c * HPC, (c + 1) * HPC)
        in_maps.append(
            {"qkv": np.stack([_pack_head(Q[h], K[h], V[h]) for h in heads])}
        )

    nc = _get_nc()
    trace = bool(int(os.environ.get("ATTN_TRACE", "0")))
    res = run_bass_kernel_spmd(
        nc, in_maps, core_ids=list(range(N_CORES)), trace=trace
    )
    _last_result = res

    out = np.empty((B * H, S, D), dtype=np.float32)
    for c in range(N_CORES):
        out[c * HPC : (c + 1) * HPC] = res.results[c]["out"]
    return out.reshape(B, H, S, D)

